# revision 48
# baseline (speedup 1.0000x reference)
"""Trainium2 Bass kernel for nn_Attention_5299989643989.

GQA attention forward (B=2, T=2048, C=1024, 16 q heads / 4 kv heads, D=64)
with value-embedding gating, rotary embedding, qk rms-norm, causal softmax.

Sharding: 8 cores = batch (2) x kv-head-group (4).  Each core computes its
4 q heads / 1 kv head end-to-end plus the Wo row-shard partial output; the
host sums the 4 partials per batch (the Wo all-reduce, done at unshard).

v2 design (bf16 datapath, ~2x over the f32r baseline):
  - All matmul operands bf16 (moving-operand dtype sets PE rate; bf16 is
    1 cycle/row at any width, f32r is 4x at <256).  Host pre-casts inputs.
  - AV computed with exp-scores as the *stationary* operand and a tiny
    [128,65] v_aug as the moving operand: cost 65 cols per (key-tile,
    q-subblock) instead of 512 per key-tile (halves AV PE time), and the
    output lands as y[q, d] so the softmax denominator is a per-partition
    scalar (plain DVE reciprocal + Pool broadcast-mul, no PE broadcast).
  - exp batched 2 key-tiles per ACT instruction ([128,2,512] PSUM bank
    pairs) to amortize the ~185ns ACT access overhead.
  - rope/rms DVE chain reads bf16 SBUF copies (Pool stages PSUM->SBUF) so
    most DVE ops run in 2x packed-16-bit mode; sigmoid gate replaced by a
    cubic Taylor poly on DVE (|z|<0.35) freeing the ACT table.
  - output staged PSUM->SBUF as bf16 on Pool, host upcasts and sums.
  - fine-grained emission interleave: attention (scores/exp/mask/AV) pair
    units are round-robined with next group's projection sub-units and
    previous group's Wo/output units, so the in-order PE stream always has
    independent work while ACT exps drain.
"""

import numpy as np

import concourse.bacc as bacc
import concourse.bass as bass
import concourse.tile as tile
from concourse import mybir
from concourse.masks import make_identity

f32 = mybir.dt.float32
bf16 = mybir.dt.bfloat16
AF = mybir.ActivationFunctionType

B, T, C = 2, 2048, 1024
N_HEAD, N_KV_HEAD, D = 16, 4, 64
HQ = N_HEAD // N_KV_HEAD  # q heads per core = 4
P = 128
NT = T // P       # 16 token chunks
KC = C // P       # 8 contraction chunks
IB = 512          # query block
NBI = T // IB     # 4 query blocks
GRP = IB // P     # 4 token chunks per query block
SC = 1.2 * 1.2 / 8.0  # folded qk scale: rms 1.2 factors * 1/sqrt(64)
H32 = D // 2


def build_program():
    nc = bacc.Bacc("TRN2", target_bir_lowering=False, debug=False, num_devices=8)

    xT = nc.dram_tensor("xT", [C, T], bf16, kind="ExternalInput")
    wr = nc.dram_tensor("wr", [C, 386], bf16, kind="ExternalInput")
    cosd = nc.dram_tensor("cosd", [T, 32], bf16, kind="ExternalInput")
    sind = nc.dram_tensor("sind", [T, 32], bf16, kind="ExternalInput")
    ve3 = nc.dram_tensor("ve3", [T, D], bf16, kind="ExternalInput")
    woT = nc.dram_tensor("woT", [2 * P, C], bf16, kind="ExternalInput")
    tri = nc.dram_tensor("tri", [P, P], bf16, kind="ExternalInput")
    out = nc.dram_tensor("out", [T, C], bf16, kind="ExternalOutput")

    with tile.TileContext(nc) as tc:
        with (
            tc.tile_pool(name="consts", bufs=1) as consts,
            tc.tile_pool(name="resid", bufs=1) as resid,
            tc.tile_pool(name="xload", bufs=3) as xload,
            tc.tile_pool(name="rot", bufs=2) as rot,
            tc.tile_pool(name="small", bufs=4) as small,
            tc.tile_pool(name="exps", bufs=6) as exps,
            tc.tile_pool(name="ysbp", bufs=2) as ysbp,
            tc.tile_pool(name="ytcp", bufs=4) as ytcp,
            tc.tile_pool(name="outsb", bufs=3) as outsb,
            tc.tile_pool(name="psmm", bufs=2, space="PSUM") as psmm,
            tc.tile_pool(name="pssc", bufs=2, space="PSUM") as pssc,
            tc.tile_pool(name="psy", bufs=2, space="PSUM") as psy,
        ):
            # ---- resident loads (batched single DMAs on SP so the ACT/SEQ
            # never serializes on HWDGE; wr+xt0 first, the rest deferred) ----
            wr_sb = consts.tile([P, KC, 386], bf16)
            xt0 = xload.tile([P, KC, IB], bf16, name="xt0", tag="xt")
            for kc in range(0, KC, 2):
                nc.sync.dma_start(
                    wr_sb[:, kc : kc + 2, :],
                    wr[kc * P : (kc + 2) * P, :].rearrange(
                        "(kc p) j -> p kc j", p=P
                    ),
                )
                nc.sync.dma_start(
                    xt0[:, kc : kc + 2, :],
                    xT[kc * P : (kc + 2) * P, 0:IB].rearrange(
                        "(kc p) t -> p kc t", p=P
                    ),
                )
            cos_sb = consts.tile([P, NT, 32], bf16)
            nc.sync.dma_start(cos_sb[:], cosd[:].rearrange("(n p) d -> p n d", p=P))
            sin_sb = consts.tile([P, NT, 32], bf16)
            nc.sync.dma_start(sin_sb[:], sind[:].rearrange("(n p) d -> p n d", p=P))
            xt1 = xload.tile([P, KC, IB], bf16, name="xt1", tag="xt")
            nc.sync.dma_start(
                xt1[:], xT[:, IB : 2 * IB].rearrange("(kc p) t -> p kc t", p=P)
            )
            state_xt1 = xt1
            ve3_sb = consts.tile([P, NT, D], bf16)
            nc.sync.dma_start(ve3_sb[:], ve3[:].rearrange("(n p) d -> p n d", p=P))
            tri_sb = consts.tile([P, P], bf16)
            nc.sync.dma_start(tri_sb[:], tri[:])
            wo1_sb = consts.tile([P, C], bf16)
            nc.sync.dma_start(wo1_sb[:], woT[0:P, :])
            wo2_sb = consts.tile([P, C], bf16)
            nc.sync.dma_start(wo2_sb[:], woT[P : 2 * P, :])
            ident = consts.tile([P, P], bf16)
            make_identity(nc, ident[:])
            rsq_k = consts.tile([P, 1], mybir.dt.uint32)
            nc.vector.memset(rsq_k[:], 0x5F3759DF)

            # ---- residents written by the kernel ----
            qT = resid.tile([P, 2, T], bf16)   # [h0|h1] rows, [h2|h3] rows
            kT2 = resid.tile([P, T], bf16)     # kT duplicated in both row halves
            v_aug = resid.tile([P, NT, D + 1], bf16)  # v plus ones column
            nc.vector.memset(v_aug[:, :, D : D + 1], 1.0)

            state = {}

            def load_x(bi):
                xt = xload.tile([P, KC, IB], bf16, name=f"xt{bi}", tag="xt")
                nc.sync.dma_start(
                    xt[:],
                    xT[:, bi * IB : (bi + 1) * IB].rearrange(
                        "(kc p) t -> p kc t", p=P
                    ),
                )
                return xt

            def gen_proj(bi, xt):
                """Projection matmuls + PSUM->SBUF staging for group bi."""
                pjg = rot.tile([P, GRP, 386], bf16, tag="pjg")
                state[("pjg", bi)] = pjg
                for tl in range(GRP):
                    pj = psmm.tile([P, 512], f32, tag="mm", name="pj")
                    for kc2 in range(0, KC, 2):
                        for kc in (kc2, kc2 + 1):
                            nc.tensor.matmul(
                                pj[:, 0:386],
                                xt[:, kc, tl * P : (tl + 1) * P],
                                wr_sb[:, kc, :],
                                start=(kc == 0),
                                stop=(kc == KC - 1),
                            )
                        yield
                    # alternate staging engine so the 2-slot psmm ring isn't
                    # paced by a single engine's copy latency
                    if tl % 2 == 0:
                        nc.scalar.copy(pjg[:, tl, :], pj[:, 0:386])
                    else:
                        nc.gpsimd.tensor_copy(pjg[:, tl, :], pj[:, 0:386])
                    yield

            def gen_dve(bi):
                """Rope + rms chain (two half-group chains), then gate/ve.
                The gate poly reads the full pjg strip, so it comes last to
                keep the rope chain unblocked in the DVE queue."""
                pjg = state.pop(("pjg", bi))
                # rms stats come straight from pjg (rope is orthogonal per
                # d-pair, so ||rope(x)|| == ||x||): squares on ACT, reduces on
                # Pool, running concurrently with the rope chain on DVE.
                qkr = rot.tile([P, GRP, 320], bf16, tag="qkr", bufs=1)
                tmp = rot.tile([P, GRP, 160], bf16, tag="tmp", bufs=1)
                qkn = rot.tile([P, GRP, 320], bf16, tag="qkn", bufs=1)
                state[("qkn", bi)] = qkn
                HG = GRP // 2
                nf = HG * 5
                sqgs = [None, None]
                for hf in range(2):
                    g0, g1 = HG * hf, HG * (hf + 1)
                    qv5 = pjg[:, g0:g1, 0:320].rearrange("p g (h d) -> p g h d", d=D)
                    ro5 = qkr[:, g0:g1, :].rearrange("p g (h d) -> p g h d", d=D)
                    t5 = tmp[:, g0:g1, :].rearrange("p g (h d) -> p g h d", d=H32)
                    cs = cos_sb[:, bi * GRP + g0 : bi * GRP + g1, :]
                    sn = sin_sb[:, bi * GRP + g0 : bi * GRP + g1, :]
                    cos5 = cs.unsqueeze(2).broadcast_to([P, HG, 5, H32])
                    sin5 = sn.unsqueeze(2).broadcast_to([P, HG, 5, H32])
                    q1 = qv5[:, :, :, 0:H32]
                    q2 = qv5[:, :, :, H32:D]
                    nc.vector.tensor_mul(ro5[:, :, :, 0:H32], q1, cos5)
                    nc.vector.tensor_mul(t5[:], q2, sin5)
                    yield
                    nc.vector.tensor_add(
                        ro5[:, :, :, 0:H32], ro5[:, :, :, 0:H32], t5[:]
                    )
                    nc.vector.tensor_mul(ro5[:, :, :, H32:D], q2, cos5)
                    yield
                    nc.vector.tensor_mul(t5[:], q1, sin5)
                    nc.vector.tensor_sub(
                        ro5[:, :, :, H32:D], ro5[:, :, :, H32:D], t5[:]
                    )
                    sqg = rot.tile([P, HG, 320], bf16, tag="sqg", bufs=2)
                    nc.scalar.square(sqg[:], qkr[:, g0:g1, :])
                    sqgs[hf] = sqg
                    yield
                for hf in range(2):
                    g0, g1 = HG * hf, HG * (hf + 1)
                    msg = small.tile([P, nf], f32, tag="msg")
                    nc.vector.reduce_sum(
                        msg[:],
                        sqgs[hf][:].rearrange("p g (h d) -> p (g h) d", d=D),
                        axis=mybir.AxisListType.X,
                    )
                    rstdg = small.tile([P, nf], f32, tag="rstdg")
                    nwt = small.tile([P, nf], f32, tag="nwt")
                    nc.vector.tensor_scalar(
                        msg[:], msg[:], 1.0 / D, 1e-6,
                        op0=mybir.AluOpType.mult, op1=mybir.AluOpType.add,
                    )
                    rstdu = rstdg[:].bitcast(mybir.dt.uint32)
                    nc.vector.tensor_scalar(
                        rstdu, msg[:].bitcast(mybir.dt.uint32), 1, None,
                        op0=mybir.AluOpType.logical_shift_right,
                    )
                    nc.vector.tensor_sub(
                        rstdu,
                        rsq_k[:].broadcast_to([P, nf]).bitcast(mybir.dt.uint32),
                        rstdu,
                    )
                    for _ in range(2):
                        nc.vector.tensor_mul(nwt[:], msg[:], rstdg[:])
                        nc.vector.tensor_mul(nwt[:], nwt[:], rstdg[:])
                        nc.vector.tensor_scalar(
                            nwt[:], nwt[:], -0.5, 1.5,
                            op0=mybir.AluOpType.mult, op1=mybir.AluOpType.add,
                        )
                        nc.vector.tensor_mul(rstdg[:], rstdg[:], nwt[:])
                    yield
                    rstdb = small.tile([P, nf], bf16, tag="rstdb")
                    nc.vector.tensor_copy(rstdb[:], rstdg[:])
                    nc.vector.tensor_mul(
                        qkn[:, g0:g1, :].rearrange("p g (h d) -> p (g h) d", d=D),
                        qkr[:, g0:g1, :].rearrange("p g (h d) -> p (g h) d", d=D),
                        rstdb[:].unsqueeze(2).broadcast_to([P, nf, D]),
                    )
                    yield

                # gate poly: sigmoid(z) ~= 0.5 + 0.25*z*(1 - z^2/12)
                # (the 3x gate factor is pre-folded into ve3 on the host)
                zc = pjg[:, :, 384]
                t0 = small.tile([P, GRP], f32, tag="t0")
                rg = small.tile([P, GRP], f32, tag="rg")
                nc.vector.tensor_mul(t0[:], zc, zc)
                nc.vector.tensor_scalar(
                    t0[:], t0[:], -1.0 / 12.0, 1.0,
                    op0=mybir.AluOpType.mult, op1=mybir.AluOpType.add,
                )
                nc.vector.tensor_mul(t0[:], t0[:], zc)
                nc.vector.tensor_scalar(
                    rg[:], t0[:], 0.25, 0.5,
                    op0=mybir.AluOpType.mult, op1=mybir.AluOpType.add,
                )
                yield
                # ve gating on Pool (v_aug bf16 = pjg_v + rg*ve3)
                vtg = small.tile([P, GRP, D], f32, tag="vtg", bufs=2)
                nc.gpsimd.tensor_mul(
                    vtg[:],
                    ve3_sb[:, bi * GRP : (bi + 1) * GRP, :],
                    rg[:].unsqueeze(2).broadcast_to([P, GRP, D]),
                )
                yield
                nc.gpsimd.tensor_add(
                    v_aug[:, bi * GRP : (bi + 1) * GRP, 0:D],
                    pjg[:, :, 320:384],
                    vtg[:],
                )
                yield

            def gen_phase1b(bi):
                """Transposes of roped/normed q,k into qT / kT2."""
                qkn = state.pop(("qkn", bi))
                for tl in range(GRP):
                    tc_ = bi * GRP + tl
                    tp = psmm.tile([P, 384], bf16, tag="mm", name="tp")
                    nc.tensor.transpose(tp[:, 0:P], qkn[:, tl, 0:128], ident[:])
                    nc.tensor.transpose(tp[:, P : 2 * P], qkn[:, tl, 128:256], ident[:])
                    nc.tensor.transpose(
                        tp[0:D, 2 * P : 3 * P], qkn[:, tl, 256:320], ident[:]
                    )
                    nc.vector.tensor_copy(
                        qT[:, :, tc_ * P : (tc_ + 1) * P],
                        tp[:, 0 : 2 * P].rearrange("p (g t) -> p g t", g=2),
                    )
                    nc.vector.tensor_copy(
                        kT2[0:D, tc_ * P : (tc_ + 1) * P], tp[0:D, 2 * P : 3 * P]
                    )
                    yield
                nc.vector.tensor_copy(
                    kT2[D:P, bi * IB : (bi + 1) * IB],
                    kT2[0:D, bi * IB : (bi + 1) * IB],
                )
                yield

            def gen_attn(bi):
                """scores -> exp -> mask -> AV -> normalize for group bi.
                Pairs flattened across heads with a lag-1 pending queue, so
                the pipeline never drains at head boundaries."""
                y_sb = ysbp.tile([P, GRP, HQ, D], bf16, tag="ysb", name="ysb")
                state[("ysb", bi)] = y_sb
                njt = GRP * (bi + 1)
                npair = njt // 2
                yps = {}

                def emit_scores(h, pi):
                    rr = D * (h % 2)
                    qTh = qT[rr : rr + D, h // 2, :]
                    jt0, jt1 = 2 * pi, 2 * pi + 1
                    lo0 = max(jt0 - GRP * bi, 0) * P
                    sp = pssc.tile([P, 2, 512], f32, tag="sc", name="sp")
                    ex = exps.tile([P, 2, 512], bf16, tag="ex", name="ex")
                    for s, jt in ((0, jt0), (1, jt1)):
                        nc.tensor.matmul(
                            sp[:, s, lo0:512],
                            kT2[rr : rr + D, jt * P : (jt + 1) * P],
                            qTh[:, bi * IB + lo0 : (bi + 1) * IB],
                            start=True, stop=True,
                        )
                    if (bi == 3 and pi in (1, 3, 5)) or (bi == 2 and pi == 2):
                        # ACT is the bottleneck in the last cycles: offload
                        # this pair's exp to DVE via a bf16 Schraudolph
                        # bit-trick (i16 = A*s + B; bits are bf16 ~exp(SC*s)).
                        nc.vector.tensor_scalar(
                            ex[:, :, lo0:512].bitcast(mybir.dt.int16),
                            sp[:, :, lo0:512],
                            33.23969374208171, 16250.9,
                            op0=mybir.AluOpType.mult, op1=mybir.AluOpType.add,
                        )
                    else:
                        nc.scalar.activation(
                            ex[:, :, lo0:512], sp[:, :, lo0:512], AF.Exp, scale=SC
                        )
                    return h, pi, ex

                def emit_av(h, pi, ex):
                    y_ps = yps[h]
                    jt0 = 2 * pi
                    for s in range(2):
                        jt = jt0 + s
                        dg = jt - GRP * bi
                        if dg >= 0:
                            nc.vector.tensor_mul(
                                ex[:, s, dg * P : (dg + 1) * P],
                                ex[:, s, dg * P : (dg + 1) * P],
                                tri_sb[:],
                            )
                    for s in range(2):
                        jt = jt0 + s
                        dg = jt - GRP * bi
                        for qb in range(max(dg, 0), GRP):
                            nc.tensor.matmul(
                                y_ps[:, qb, :],
                                ex[:, s, qb * P : (qb + 1) * P],
                                v_aug[:, jt, :],
                                start=(pi == 0 and s == 0 and qb == max(dg, 0)),
                                stop=(jt == njt - 1 and qb == GRP - 1),
                                skip_group_check=True,
                            )
                    if pi == npair - 1:
                        # normalize head h into y_sb; frees the psy slot.
                        # (split per qb-pair in the last group to shorten the
                        # tail chain into gen_wo)
                        den = small.tile([P, GRP], f32, tag="den")
                        nc.gpsimd.tensor_copy(den[:], y_ps[:, :, D])
                        rec = small.tile([P, GRP], f32, tag="rec")
                        nc.vector.reciprocal_approx_fast(rec[:], den[:])
                        qsp = 2 if bi == NBI - 1 else GRP
                        for q0 in range(0, GRP, qsp):
                            nc.gpsimd.tensor_mul(
                                y_sb[:, q0 : q0 + qsp, h, :],
                                y_ps[:, q0 : q0 + qsp, 0:D],
                                rec[:, q0 : q0 + qsp]
                                .unsqueeze(2)
                                .broadcast_to([P, qsp, D]),
                            )
                        del yps[h]

                pending = []
                for h in range(HQ):
                    yps[h] = psy.tile([P, GRP, D + 1], f32, tag="y", name="yps")
                    for pi in range(npair):
                        pending.append(emit_scores(h, pi))
                        if len(pending) > 3:
                            emit_av(*pending.pop(0))
                        yield
                while pending:
                    emit_av(*pending.pop(0))
                    yield

            def gen_wo(bi, tail=False):
                """y transposes + Wo row-shard + staging + DMA for group bi.
                All transposes first so the po matmul run is never starved."""
                y_sb = state.pop(("ysb", bi))
                ytcs = []
                for qb in range(GRP):
                    ytp = psmm.tile([P, 256], bf16, tag="mm", name="ytp")
                    nc.tensor.transpose(
                        ytp[:, 0:P],
                        y_sb[:, qb, 0:2, :].rearrange("p h d -> p (h d)"),
                        ident[:],
                    )
                    nc.tensor.transpose(
                        ytp[:, P : 2 * P],
                        y_sb[:, qb, 2:4, :].rearrange("p h d -> p (h d)"),
                        ident[:],
                    )
                    ytc = ytcp.tile([P, 256], bf16, tag="ytc", name="ytc")
                    nc.vector.tensor_copy(ytc[:], ytp[:])
                    ytcs.append(ytc)
                    yield
                for qb in range(GRP):
                    tc_ = bi * GRP + qb
                    ytc = ytcs[qb]
                    for cb in range(2):
                        po = psmm.tile([P, 512], f32, tag="mm", name="po")
                        nc.tensor.matmul(
                            po[:],
                            ytc[:, 0:P],
                            wo1_sb[:, cb * 512 : (cb + 1) * 512],
                            start=True, stop=False,
                        )
                        nc.tensor.matmul(
                            po[:],
                            ytc[:, P : 2 * P],
                            wo2_sb[:, cb * 512 : (cb + 1) * 512],
                            start=False, stop=True,
                        )
                        ob = outsb.tile([P, 512], bf16, tag="ob", name="ob")
                        eng = (qb * 2 + cb) % 3 if tail else cb
                        if eng == 0:
                            nc.gpsimd.tensor_copy(ob[:], po[:])
                        elif eng == 1:
                            nc.vector.tensor_copy(ob[:], po[:])
                        else:
                            nc.scalar.copy(ob[:], po[:])
                        nc.sync.dma_start(
                            out[tc_ * P : (tc_ + 1) * P, cb * 512 : (cb + 1) * 512],
                            ob[:],
                        )
                        yield

            # ---- cycle driver with pacing modes ----
            def attn_units(bi):
                return HQ * ((bi + 1) * GRP // 2) + 1

            PROJ_UNITS = GRP * 5
            DVE_UNITS = 16
            WO_UNITS = GRP * 3
            P1B_UNITS = GRP + 1

            def drain(gen):
                for _ in gen:
                    pass

            def run_cycle(main, n_main, sides):
                """sides: (gen, n_units, mode); mode 'front' finishes ~55%
                through main, 'even' spreads, 'back' starts at ~55%."""
                prog = []
                for g, n, mode in sides:
                    if mode == "front":
                        prog.append([g, n / (0.55 * n_main), 0.0, 0])
                    elif mode == "back":
                        prog.append([g, n / (0.45 * n_main), 0.0, int(0.55 * n_main)])
                    else:
                        prog.append([g, n / n_main, 0.0, 0])
                step = 0
                while True:
                    try:
                        next(main)
                    except StopIteration:
                        break
                    step += 1
                    for s in prog:
                        if s[0] is None or step < s[3]:
                            continue
                        s[2] += s[1]
                        while s[2] >= 1.0:
                            try:
                                next(s[0])
                            except StopIteration:
                                s[0] = None
                                break
                            s[2] -= 1.0
                for s in prog:
                    if s[0] is not None:
                        drain(s[0])

            # ---- pipeline ----
            # prologue: proj(0) | dve(0) with proj(1) filling the PE | p1b(0)
            state[("xt", 1)] = state_xt1
            state[("xt", 2)] = load_x(2)
            drain(gen_proj(0, xt0))
            run_cycle(
                gen_dve(0), DVE_UNITS,
                [(gen_proj(1, state.pop(("xt", 1))), PROJ_UNITS, "even")],
            )
            drain(gen_phase1b(0))
            state[("xt", 3)] = load_x(3)

            # cycle 0: attn(0) + dve(1)/proj(2) + p1b(1)
            run_cycle(
                gen_attn(0), attn_units(0),
                [
                    (gen_dve(1), DVE_UNITS, "front"),
                    (gen_proj(2, state.pop(("xt", 2))), PROJ_UNITS, "even"),
                    (gen_phase1b(1), P1B_UNITS, "back"),
                ],
            )
            # cycle 1: attn(1) + dve(2)/proj(3) + wo(0) + p1b(2)
            run_cycle(
                gen_attn(1), attn_units(1),
                [
                    (gen_dve(2), DVE_UNITS, "front"),
                    (gen_proj(3, state.pop(("xt", 3))), PROJ_UNITS, "even"),
                    (gen_wo(0), WO_UNITS, "even"),
                    (gen_phase1b(2), P1B_UNITS, "back"),
                ],
            )
            # cycle 2: attn(2) + dve(3) + wo(1) + p1b(3)
            run_cycle(
                gen_attn(2), attn_units(2),
                [
                    (gen_dve(3), DVE_UNITS, "front"),
                    (gen_wo(1), WO_UNITS, "even"),
                    (gen_phase1b(3), P1B_UNITS, "back"),
                ],
            )
            # cycle 3: attn(3) + wo(2)
            run_cycle(
                gen_attn(3), attn_units(3),
                [(gen_wo(2), WO_UNITS, "front")],
            )
            drain(gen_wo(NBI - 1, tail=True))

    nc.compile()
    return nc


def make_core_inputs(x, ve, cos, sin, Wq, Wk, Wv, Wo, Wg):
    """Slice full inputs into the 8 per-core input maps (b-major, then group),
    pre-cast to bf16."""
    import ml_dtypes

    bf = ml_dtypes.bfloat16
    cosf = np.ascontiguousarray(cos[0, :, 0, :], dtype=np.float32)  # [T, 32]
    sinf = np.ascontiguousarray(sin[0, :, 0, :], dtype=np.float32)
    tri = (np.arange(P)[:, None] <= np.arange(P)[None, :]).astype(bf)
    in_maps = []
    for c in range(8):
        b, g = c // N_KV_HEAD, c % N_KV_HEAD
        xTc = np.ascontiguousarray(x[b].T).astype(bf)  # [C, T]
        wq = Wq[g * 256 : (g + 1) * 256, :]           # [256, C]
        wk = Wk[g * D : (g + 1) * D, :]               # [64, C]
        wv = Wv[g * D : (g + 1) * D, :]
        wg_col = np.zeros((C, 1), np.float32)
        wg_col[:12, 0] = Wg[g]
        wrc = np.concatenate(
            [wq.T, wk.T, wv.T, wg_col, np.zeros((C, 1), np.float32)], axis=1
        ).astype(bf)                                  # [C, 386]
        ve3 = (3.0 * ve[b, :, g * D : (g + 1) * D]).astype(bf)  # [T, 64]
        woTc = np.ascontiguousarray(Wo[:, g * 256 : (g + 1) * 256].T).astype(bf)
        in_maps.append(
            {
                "xT": xTc,
                "wr": np.ascontiguousarray(wrc),
                "cosd": cosf.astype(bf),
                "sind": sinf.astype(bf),
                "ve3": np.ascontiguousarray(ve3),
                "woT": woTc,
                "tri": tri,
            }
        )
    return in_maps


_PROGRAM = None


def kernel(x, ve, cos, sin, Wq, Wk, Wv, Wo, Wg, _trace=False):
    from concourse.bass_utils import run_bass_kernel_spmd

    x, ve, cos, sin, Wq, Wk, Wv, Wo, Wg = (
        np.asarray(a, dtype=np.float32)
        for a in (x, ve, cos, sin, Wq, Wk, Wv, Wo, Wg)
    )
    global _PROGRAM
    if _PROGRAM is None:
        _PROGRAM = build_program()
    nc = _PROGRAM
    in_maps = make_core_inputs(x, ve, cos, sin, Wq, Wk, Wv, Wo, Wg)
    res = run_bass_kernel_spmd(nc, in_maps, list(range(8)), trace=_trace)
    outs = [np.asarray(r["out"], np.float32) for r in res.results]
    full = np.zeros((B, T, C), np.float32)
    for c in range(8):
        full[c // N_KV_HEAD] += outs[c]
    if _trace:
        kernel.last_results = res
    return full


# revision 49
# speedup vs baseline: 1.0183x; 1.0183x over previous
"""Trainium2 Bass kernel for nn_Attention_5299989643989.

GQA attention forward (B=2, T=2048, C=1024, 16 q heads / 4 kv heads, D=64)
with value-embedding gating, rotary embedding, qk rms-norm, causal softmax.

Sharding: 8 cores = batch (2) x kv-head-group (4).  Each core computes its
4 q heads / 1 kv head end-to-end plus the Wo row-shard partial output; the
host sums the 4 partials per batch (the Wo all-reduce, done at unshard).

v2 design (bf16 datapath, ~2x over the f32r baseline):
  - All matmul operands bf16 (moving-operand dtype sets PE rate; bf16 is
    1 cycle/row at any width, f32r is 4x at <256).  Host pre-casts inputs.
  - AV computed with exp-scores as the *stationary* operand and a tiny
    [128,65] v_aug as the moving operand: cost 65 cols per (key-tile,
    q-subblock) instead of 512 per key-tile (halves AV PE time), and the
    output lands as y[q, d] so the softmax denominator is a per-partition
    scalar (plain DVE reciprocal + Pool broadcast-mul, no PE broadcast).
  - exp batched 2 key-tiles per ACT instruction ([128,2,512] PSUM bank
    pairs) to amortize the ~185ns ACT access overhead.
  - rope/rms DVE chain reads bf16 SBUF copies (Pool stages PSUM->SBUF) so
    most DVE ops run in 2x packed-16-bit mode; sigmoid gate replaced by a
    cubic Taylor poly on DVE (|z|<0.35) freeing the ACT table.
  - output staged PSUM->SBUF as bf16 on Pool, host upcasts and sums.
  - fine-grained emission interleave: attention (scores/exp/mask/AV) pair
    units are round-robined with next group's projection sub-units and
    previous group's Wo/output units, so the in-order PE stream always has
    independent work while ACT exps drain.
"""

import numpy as np

import concourse.bacc as bacc
import concourse.bass as bass
import concourse.tile as tile
from concourse import mybir
from concourse.masks import make_identity

f32 = mybir.dt.float32
bf16 = mybir.dt.bfloat16
AF = mybir.ActivationFunctionType

B, T, C = 2, 2048, 1024
N_HEAD, N_KV_HEAD, D = 16, 4, 64
HQ = N_HEAD // N_KV_HEAD  # q heads per core = 4
P = 128
NT = T // P       # 16 token chunks
KC = C // P       # 8 contraction chunks
IB = 512          # query block
NBI = T // IB     # 4 query blocks
GRP = IB // P     # 4 token chunks per query block
SC = 1.2 * 1.2 / 8.0  # folded qk scale: rms 1.2 factors * 1/sqrt(64)
H32 = D // 2


def build_program():
    nc = bacc.Bacc("TRN2", target_bir_lowering=False, debug=False, num_devices=8)

    xT = nc.dram_tensor("xT", [C, T], bf16, kind="ExternalInput")
    wr = nc.dram_tensor("wr", [C, 386], bf16, kind="ExternalInput")
    cosd = nc.dram_tensor("cosd", [T, 32], bf16, kind="ExternalInput")
    sind = nc.dram_tensor("sind", [T, 32], bf16, kind="ExternalInput")
    ve3 = nc.dram_tensor("ve3", [T, D], bf16, kind="ExternalInput")
    woT = nc.dram_tensor("woT", [2 * P, C], bf16, kind="ExternalInput")
    tri = nc.dram_tensor("tri", [P, P], bf16, kind="ExternalInput")
    out = nc.dram_tensor("out", [T, C], bf16, kind="ExternalOutput")

    with tile.TileContext(nc) as tc:
        with (
            tc.tile_pool(name="consts", bufs=1) as consts,
            tc.tile_pool(name="resid", bufs=1) as resid,
            tc.tile_pool(name="xload", bufs=3) as xload,
            tc.tile_pool(name="rot", bufs=2) as rot,
            tc.tile_pool(name="small", bufs=4) as small,
            tc.tile_pool(name="exps", bufs=6) as exps,
            tc.tile_pool(name="ysbp", bufs=2) as ysbp,
            tc.tile_pool(name="ytcp", bufs=4) as ytcp,
            tc.tile_pool(name="outsb", bufs=3) as outsb,
            tc.tile_pool(name="psmm", bufs=2, space="PSUM") as psmm,
            tc.tile_pool(name="pssc", bufs=2, space="PSUM") as pssc,
            tc.tile_pool(name="psy", bufs=2, space="PSUM") as psy,
        ):
            # ---- resident loads (batched single DMAs on SP so the ACT/SEQ
            # never serializes on HWDGE; wr+xt0 first, the rest deferred) ----
            wr_sb = consts.tile([P, KC, 386], bf16)
            xt0 = xload.tile([P, KC, IB], bf16, name="xt0", tag="xt")
            for kc in range(0, KC, 2):
                nc.sync.dma_start(
                    wr_sb[:, kc : kc + 2, :],
                    wr[kc * P : (kc + 2) * P, :].rearrange(
                        "(kc p) j -> p kc j", p=P
                    ),
                )
                nc.sync.dma_start(
                    xt0[:, kc : kc + 2, :],
                    xT[kc * P : (kc + 2) * P, 0:IB].rearrange(
                        "(kc p) t -> p kc t", p=P
                    ),
                )
            cos_sb = consts.tile([P, NT, 32], bf16)
            nc.sync.dma_start(cos_sb[:], cosd[:].rearrange("(n p) d -> p n d", p=P))
            sin_sb = consts.tile([P, NT, 32], bf16)
            nc.sync.dma_start(sin_sb[:], sind[:].rearrange("(n p) d -> p n d", p=P))
            xt1 = xload.tile([P, KC, IB], bf16, name="xt1", tag="xt")
            nc.sync.dma_start(
                xt1[:], xT[:, IB : 2 * IB].rearrange("(kc p) t -> p kc t", p=P)
            )
            state_xt1 = xt1
            ve3_sb = consts.tile([P, NT, D], bf16)
            nc.sync.dma_start(ve3_sb[:], ve3[:].rearrange("(n p) d -> p n d", p=P))
            tri_sb = consts.tile([P, P], bf16)
            nc.sync.dma_start(tri_sb[:], tri[:])
            wo1_sb = consts.tile([P, C], bf16)
            nc.sync.dma_start(wo1_sb[:], woT[0:P, :])
            wo2_sb = consts.tile([P, C], bf16)
            nc.sync.dma_start(wo2_sb[:], woT[P : 2 * P, :])
            ident = consts.tile([P, P], bf16)
            make_identity(nc, ident[:])
            rsq_k = consts.tile([P, 1], mybir.dt.uint32)
            nc.vector.memset(rsq_k[:], 0x5F3759DF)

            # ---- residents written by the kernel ----
            qT = resid.tile([P, 2, T], bf16)   # [h0|h1] rows, [h2|h3] rows
            kT2 = resid.tile([P, T], bf16)     # kT duplicated in both row halves
            v_aug = resid.tile([P, NT, D + 1], bf16)  # v plus ones column
            nc.vector.memset(v_aug[:, :, D : D + 1], 1.0)

            state = {}

            def load_x(bi):
                xt = xload.tile([P, KC, IB], bf16, name=f"xt{bi}", tag="xt")
                nc.sync.dma_start(
                    xt[:],
                    xT[:, bi * IB : (bi + 1) * IB].rearrange(
                        "(kc p) t -> p kc t", p=P
                    ),
                )
                return xt

            def gen_proj(bi, xt):
                """Projection matmuls + PSUM->SBUF staging for group bi."""
                pjg = rot.tile([P, GRP, 386], bf16, tag="pjg")
                state[("pjg", bi)] = pjg
                for tl in range(GRP):
                    pj = psmm.tile([P, 512], f32, tag="mm", name="pj")
                    for kc2 in range(0, KC, 2):
                        for kc in (kc2, kc2 + 1):
                            nc.tensor.matmul(
                                pj[:, 0:386],
                                xt[:, kc, tl * P : (tl + 1) * P],
                                wr_sb[:, kc, :],
                                start=(kc == 0),
                                stop=(kc == KC - 1),
                            )
                        yield
                    # alternate staging engine so the 2-slot psmm ring isn't
                    # paced by a single engine's copy latency
                    if tl % 2 == 0:
                        nc.scalar.copy(pjg[:, tl, :], pj[:, 0:386])
                    else:
                        nc.gpsimd.tensor_copy(pjg[:, tl, :], pj[:, 0:386])
                    yield

            def gen_dve(bi):
                """Rope + rms chain (two half-group chains), then gate/ve.
                The gate poly reads the full pjg strip, so it comes last to
                keep the rope chain unblocked in the DVE queue."""
                pjg = state.pop(("pjg", bi))
                # rms stats come straight from pjg (rope is orthogonal per
                # d-pair, so ||rope(x)|| == ||x||): squares on ACT, reduces on
                # Pool, running concurrently with the rope chain on DVE.
                qkr = rot.tile([P, GRP, 320], bf16, tag="qkr", bufs=1)
                tmp = rot.tile([P, GRP, 160], bf16, tag="tmp", bufs=1)
                qkn = rot.tile([P, GRP, 320], bf16, tag="qkn", bufs=1)
                state[("qkn", bi)] = qkn
                HG = GRP // 2
                nf = HG * 5
                sqgs = [None, None]
                for hf in range(2):
                    g0, g1 = HG * hf, HG * (hf + 1)
                    qv5 = pjg[:, g0:g1, 0:320].rearrange("p g (h d) -> p g h d", d=D)
                    ro5 = qkr[:, g0:g1, :].rearrange("p g (h d) -> p g h d", d=D)
                    t5 = tmp[:, g0:g1, :].rearrange("p g (h d) -> p g h d", d=H32)
                    cs = cos_sb[:, bi * GRP + g0 : bi * GRP + g1, :]
                    sn = sin_sb[:, bi * GRP + g0 : bi * GRP + g1, :]
                    cos5 = cs.unsqueeze(2).broadcast_to([P, HG, 5, H32])
                    sin5 = sn.unsqueeze(2).broadcast_to([P, HG, 5, H32])
                    q1 = qv5[:, :, :, 0:H32]
                    q2 = qv5[:, :, :, H32:D]
                    nc.vector.tensor_mul(ro5[:, :, :, 0:H32], q1, cos5)
                    nc.vector.tensor_mul(t5[:], q2, sin5)
                    yield
                    nc.vector.tensor_add(
                        ro5[:, :, :, 0:H32], ro5[:, :, :, 0:H32], t5[:]
                    )
                    nc.vector.tensor_mul(ro5[:, :, :, H32:D], q2, cos5)
                    yield
                    nc.vector.tensor_mul(t5[:], q1, sin5)
                    nc.vector.tensor_sub(
                        ro5[:, :, :, H32:D], ro5[:, :, :, H32:D], t5[:]
                    )
                    sqg = rot.tile([P, HG, 320], bf16, tag="sqg", bufs=2)
                    nc.vector.tensor_mul(sqg[:], qkr[:, g0:g1, :], qkr[:, g0:g1, :])
                    sqgs[hf] = sqg
                    yield
                for hf in range(2):
                    g0, g1 = HG * hf, HG * (hf + 1)
                    msg = small.tile([P, nf], f32, tag="msg")
                    nc.vector.reduce_sum(
                        msg[:],
                        sqgs[hf][:].rearrange("p g (h d) -> p (g h) d", d=D),
                        axis=mybir.AxisListType.X,
                    )
                    rstdg = small.tile([P, nf], f32, tag="rstdg")
                    nwt = small.tile([P, nf], f32, tag="nwt")
                    nc.vector.tensor_scalar(
                        msg[:], msg[:], 1.0 / D, 1e-6,
                        op0=mybir.AluOpType.mult, op1=mybir.AluOpType.add,
                    )
                    rstdu = rstdg[:].bitcast(mybir.dt.uint32)
                    nc.vector.tensor_scalar(
                        rstdu, msg[:].bitcast(mybir.dt.uint32), 1, None,
                        op0=mybir.AluOpType.logical_shift_right,
                    )
                    nc.vector.tensor_sub(
                        rstdu,
                        rsq_k[:].broadcast_to([P, nf]).bitcast(mybir.dt.uint32),
                        rstdu,
                    )
                    for _ in range(2):
                        nc.vector.tensor_mul(nwt[:], msg[:], rstdg[:])
                        nc.vector.tensor_mul(nwt[:], nwt[:], rstdg[:])
                        nc.vector.tensor_scalar(
                            nwt[:], nwt[:], -0.5, 1.5,
                            op0=mybir.AluOpType.mult, op1=mybir.AluOpType.add,
                        )
                        nc.vector.tensor_mul(rstdg[:], rstdg[:], nwt[:])
                    yield
                    rstdb = small.tile([P, nf], bf16, tag="rstdb")
                    nc.vector.tensor_copy(rstdb[:], rstdg[:])
                    nc.vector.tensor_mul(
                        qkn[:, g0:g1, :].rearrange("p g (h d) -> p (g h) d", d=D),
                        qkr[:, g0:g1, :].rearrange("p g (h d) -> p (g h) d", d=D),
                        rstdb[:].unsqueeze(2).broadcast_to([P, nf, D]),
                    )
                    yield

                # gate poly: sigmoid(z) ~= 0.5 + 0.25*z*(1 - z^2/12)
                # (the 3x gate factor is pre-folded into ve3 on the host)
                zc = pjg[:, :, 384]
                t0 = small.tile([P, GRP], f32, tag="t0")
                rg = small.tile([P, GRP], f32, tag="rg")
                nc.vector.tensor_mul(t0[:], zc, zc)
                nc.vector.tensor_scalar(
                    t0[:], t0[:], -1.0 / 12.0, 1.0,
                    op0=mybir.AluOpType.mult, op1=mybir.AluOpType.add,
                )
                nc.vector.tensor_mul(t0[:], t0[:], zc)
                nc.vector.tensor_scalar(
                    rg[:], t0[:], 0.25, 0.5,
                    op0=mybir.AluOpType.mult, op1=mybir.AluOpType.add,
                )
                yield
                # ve gating on Pool (v_aug bf16 = pjg_v + rg*ve3)
                vtg = small.tile([P, GRP, D], f32, tag="vtg", bufs=2)
                nc.gpsimd.tensor_mul(
                    vtg[:],
                    ve3_sb[:, bi * GRP : (bi + 1) * GRP, :],
                    rg[:].unsqueeze(2).broadcast_to([P, GRP, D]),
                )
                yield
                nc.gpsimd.tensor_add(
                    v_aug[:, bi * GRP : (bi + 1) * GRP, 0:D],
                    pjg[:, :, 320:384],
                    vtg[:],
                )
                yield

            def gen_phase1b(bi):
                """Transposes of roped/normed q,k into qT / kT2."""
                qkn = state.pop(("qkn", bi))
                for tl in range(GRP):
                    tc_ = bi * GRP + tl
                    tp = psmm.tile([P, 384], bf16, tag="mm", name="tp")
                    nc.tensor.transpose(tp[:, 0:P], qkn[:, tl, 0:128], ident[:])
                    nc.tensor.transpose(tp[:, P : 2 * P], qkn[:, tl, 128:256], ident[:])
                    nc.tensor.transpose(
                        tp[0:D, 2 * P : 3 * P], qkn[:, tl, 256:320], ident[:]
                    )
                    nc.vector.tensor_copy(
                        qT[:, :, tc_ * P : (tc_ + 1) * P],
                        tp[:, 0 : 2 * P].rearrange("p (g t) -> p g t", g=2),
                    )
                    nc.vector.tensor_copy(
                        kT2[0:D, tc_ * P : (tc_ + 1) * P], tp[0:D, 2 * P : 3 * P]
                    )
                    yield
                nc.vector.tensor_copy(
                    kT2[D:P, bi * IB : (bi + 1) * IB],
                    kT2[0:D, bi * IB : (bi + 1) * IB],
                )
                yield

            def gen_attn(bi):
                """scores -> exp -> mask -> AV -> normalize for group bi.
                Pairs flattened across heads with a lag-1 pending queue, so
                the pipeline never drains at head boundaries."""
                y_sb = ysbp.tile([P, GRP, HQ, D], bf16, tag="ysb", name="ysb")
                state[("ysb", bi)] = y_sb
                njt = GRP * (bi + 1)
                npair = njt // 2
                yps = {}

                def emit_scores(h, pi):
                    rr = D * (h % 2)
                    qTh = qT[rr : rr + D, h // 2, :]
                    jt0, jt1 = 2 * pi, 2 * pi + 1
                    lo0 = max(jt0 - GRP * bi, 0) * P
                    sp = pssc.tile([P, 2, 512], f32, tag="sc", name="sp")
                    ex = exps.tile([P, 2, 512], bf16, tag="ex", name="ex")
                    for s, jt in ((0, jt0), (1, jt1)):
                        nc.tensor.matmul(
                            sp[:, s, lo0:512],
                            kT2[rr : rr + D, jt * P : (jt + 1) * P],
                            qTh[:, bi * IB + lo0 : (bi + 1) * IB],
                            start=True, stop=True,
                        )
                    if (bi == 3 and pi in (1, 3, 5)) or (bi == 2 and pi == 2):
                        # ACT is the bottleneck in the last cycles: offload
                        # this pair's exp to DVE via a bf16 Schraudolph
                        # bit-trick (i16 = A*s + B; bits are bf16 ~exp(SC*s)).
                        nc.vector.tensor_scalar(
                            ex[:, :, lo0:512].bitcast(mybir.dt.int16),
                            sp[:, :, lo0:512],
                            33.23969374208171, 16250.9,
                            op0=mybir.AluOpType.mult, op1=mybir.AluOpType.add,
                        )
                    else:
                        nc.scalar.activation(
                            ex[:, :, lo0:512], sp[:, :, lo0:512], AF.Exp, scale=SC
                        )
                    return h, pi, ex

                def emit_av(h, pi, ex):
                    y_ps = yps[h]
                    jt0 = 2 * pi
                    for s in range(2):
                        jt = jt0 + s
                        dg = jt - GRP * bi
                        if dg >= 0:
                            nc.vector.tensor_mul(
                                ex[:, s, dg * P : (dg + 1) * P],
                                ex[:, s, dg * P : (dg + 1) * P],
                                tri_sb[:],
                            )
                    for s in range(2):
                        jt = jt0 + s
                        dg = jt - GRP * bi
                        for qb in range(max(dg, 0), GRP):
                            nc.tensor.matmul(
                                y_ps[:, qb, :],
                                ex[:, s, qb * P : (qb + 1) * P],
                                v_aug[:, jt, :],
                                start=(pi == 0 and s == 0 and qb == max(dg, 0)),
                                stop=(jt == njt - 1 and qb == GRP - 1),
                                skip_group_check=True,
                            )
                    if pi == npair - 1:
                        # normalize head h into y_sb; frees the psy slot.
                        # (split per qb-pair in the last group to shorten the
                        # tail chain into gen_wo)
                        den = small.tile([P, GRP], f32, tag="den")
                        nc.gpsimd.tensor_copy(den[:], y_ps[:, :, D])
                        rec = small.tile([P, GRP], f32, tag="rec")
                        nc.vector.reciprocal_approx_fast(rec[:], den[:])
                        qsp = 2 if bi == NBI - 1 else GRP
                        for q0 in range(0, GRP, qsp):
                            nc.gpsimd.tensor_mul(
                                y_sb[:, q0 : q0 + qsp, h, :],
                                y_ps[:, q0 : q0 + qsp, 0:D],
                                rec[:, q0 : q0 + qsp]
                                .unsqueeze(2)
                                .broadcast_to([P, qsp, D]),
                            )
                        del yps[h]

                pending = []
                for h in range(HQ):
                    yps[h] = psy.tile([P, GRP, D + 1], f32, tag="y", name="yps")
                    for pi in range(npair):
                        pending.append(emit_scores(h, pi))
                        if len(pending) > 3:
                            emit_av(*pending.pop(0))
                        yield
                while pending:
                    emit_av(*pending.pop(0))
                    yield

            def gen_wo(bi, tail=False):
                """y transposes + Wo row-shard + staging + DMA for group bi.
                All transposes first so the po matmul run is never starved."""
                y_sb = state.pop(("ysb", bi))
                ytcs = []
                for qb in range(GRP):
                    ytp = psmm.tile([P, 256], bf16, tag="mm", name="ytp")
                    nc.tensor.transpose(
                        ytp[:, 0:P],
                        y_sb[:, qb, 0:2, :].rearrange("p h d -> p (h d)"),
                        ident[:],
                    )
                    nc.tensor.transpose(
                        ytp[:, P : 2 * P],
                        y_sb[:, qb, 2:4, :].rearrange("p h d -> p (h d)"),
                        ident[:],
                    )
                    ytc = ytcp.tile([P, 256], bf16, tag="ytc", name="ytc")
                    nc.vector.tensor_copy(ytc[:], ytp[:])
                    ytcs.append(ytc)
                    yield
                for qb in range(GRP):
                    tc_ = bi * GRP + qb
                    ytc = ytcs[qb]
                    for cb in range(2):
                        po = psmm.tile([P, 512], f32, tag="mm", name="po")
                        nc.tensor.matmul(
                            po[:],
                            ytc[:, 0:P],
                            wo1_sb[:, cb * 512 : (cb + 1) * 512],
                            start=True, stop=False,
                        )
                        nc.tensor.matmul(
                            po[:],
                            ytc[:, P : 2 * P],
                            wo2_sb[:, cb * 512 : (cb + 1) * 512],
                            start=False, stop=True,
                        )
                        ob = outsb.tile([P, 512], bf16, tag="ob", name="ob")
                        eng = (qb * 2 + cb) % 3 if tail else cb
                        if eng == 0:
                            nc.gpsimd.tensor_copy(ob[:], po[:])
                        elif eng == 1:
                            nc.vector.tensor_copy(ob[:], po[:])
                        else:
                            nc.scalar.copy(ob[:], po[:])
                        nc.sync.dma_start(
                            out[tc_ * P : (tc_ + 1) * P, cb * 512 : (cb + 1) * 512],
                            ob[:],
                        )
                        yield

            # ---- cycle driver with pacing modes ----
            def attn_units(bi):
                return HQ * ((bi + 1) * GRP // 2) + 1

            PROJ_UNITS = GRP * 5
            DVE_UNITS = 16
            WO_UNITS = GRP * 3
            P1B_UNITS = GRP + 1

            def drain(gen):
                for _ in gen:
                    pass

            def run_cycle(main, n_main, sides):
                """sides: (gen, n_units, mode); mode 'front' finishes ~55%
                through main, 'even' spreads, 'back' starts at ~55%."""
                prog = []
                for g, n, mode in sides:
                    if mode == "front":
                        prog.append([g, n / (0.55 * n_main), 0.0, 0])
                    elif mode == "back":
                        prog.append([g, n / (0.45 * n_main), 0.0, int(0.55 * n_main)])
                    else:
                        prog.append([g, n / n_main, 0.0, 0])
                step = 0
                while True:
                    try:
                        next(main)
                    except StopIteration:
                        break
                    step += 1
                    for s in prog:
                        if s[0] is None or step < s[3]:
                            continue
                        s[2] += s[1]
                        while s[2] >= 1.0:
                            try:
                                next(s[0])
                            except StopIteration:
                                s[0] = None
                                break
                            s[2] -= 1.0
                for s in prog:
                    if s[0] is not None:
                        drain(s[0])

            # ---- pipeline ----
            # prologue: proj(0) | dve(0) with proj(1) filling the PE | p1b(0)
            state[("xt", 1)] = state_xt1
            state[("xt", 2)] = load_x(2)
            drain(gen_proj(0, xt0))
            run_cycle(
                gen_dve(0), DVE_UNITS,
                [(gen_proj(1, state.pop(("xt", 1))), PROJ_UNITS, "even")],
            )
            drain(gen_phase1b(0))
            state[("xt", 3)] = load_x(3)

            # cycle 0: attn(0) + dve(1)/proj(2) + p1b(1)
            run_cycle(
                gen_attn(0), attn_units(0),
                [
                    (gen_dve(1), DVE_UNITS, "front"),
                    (gen_proj(2, state.pop(("xt", 2))), PROJ_UNITS, "even"),
                    (gen_phase1b(1), P1B_UNITS, "back"),
                ],
            )
            # cycle 1: attn(1) + dve(2)/proj(3) + wo(0) + p1b(2)
            run_cycle(
                gen_attn(1), attn_units(1),
                [
                    (gen_dve(2), DVE_UNITS, "front"),
                    (gen_proj(3, state.pop(("xt", 3))), PROJ_UNITS, "even"),
                    (gen_wo(0), WO_UNITS, "even"),
                    (gen_phase1b(2), P1B_UNITS, "back"),
                ],
            )
            # cycle 2: attn(2) + dve(3) + wo(1) + p1b(3)
            run_cycle(
                gen_attn(2), attn_units(2),
                [
                    (gen_dve(3), DVE_UNITS, "front"),
                    (gen_wo(1), WO_UNITS, "even"),
                    (gen_phase1b(3), P1B_UNITS, "back"),
                ],
            )
            # cycle 3: attn(3) + wo(2)
            run_cycle(
                gen_attn(3), attn_units(3),
                [(gen_wo(2), WO_UNITS, "front")],
            )
            drain(gen_wo(NBI - 1, tail=True))

    nc.compile()
    return nc


def make_core_inputs(x, ve, cos, sin, Wq, Wk, Wv, Wo, Wg):
    """Slice full inputs into the 8 per-core input maps (b-major, then group),
    pre-cast to bf16."""
    import ml_dtypes

    bf = ml_dtypes.bfloat16
    cosf = np.ascontiguousarray(cos[0, :, 0, :], dtype=np.float32)  # [T, 32]
    sinf = np.ascontiguousarray(sin[0, :, 0, :], dtype=np.float32)
    tri = (np.arange(P)[:, None] <= np.arange(P)[None, :]).astype(bf)
    in_maps = []
    for c in range(8):
        b, g = c // N_KV_HEAD, c % N_KV_HEAD
        xTc = np.ascontiguousarray(x[b].T).astype(bf)  # [C, T]
        wq = Wq[g * 256 : (g + 1) * 256, :]           # [256, C]
        wk = Wk[g * D : (g + 1) * D, :]               # [64, C]
        wv = Wv[g * D : (g + 1) * D, :]
        wg_col = np.zeros((C, 1), np.float32)
        wg_col[:12, 0] = Wg[g]
        wrc = np.concatenate(
            [wq.T, wk.T, wv.T, wg_col, np.zeros((C, 1), np.float32)], axis=1
        ).astype(bf)                                  # [C, 386]
        ve3 = (3.0 * ve[b, :, g * D : (g + 1) * D]).astype(bf)  # [T, 64]
        woTc = np.ascontiguousarray(Wo[:, g * 256 : (g + 1) * 256].T).astype(bf)
        in_maps.append(
            {
                "xT": xTc,
                "wr": np.ascontiguousarray(wrc),
                "cosd": cosf.astype(bf),
                "sind": sinf.astype(bf),
                "ve3": np.ascontiguousarray(ve3),
                "woT": woTc,
                "tri": tri,
            }
        )
    return in_maps


_PROGRAM = None


def kernel(x, ve, cos, sin, Wq, Wk, Wv, Wo, Wg, _trace=False):
    from concourse.bass_utils import run_bass_kernel_spmd

    x, ve, cos, sin, Wq, Wk, Wv, Wo, Wg = (
        np.asarray(a, dtype=np.float32)
        for a in (x, ve, cos, sin, Wq, Wk, Wv, Wo, Wg)
    )
    global _PROGRAM
    if _PROGRAM is None:
        _PROGRAM = build_program()
    nc = _PROGRAM
    in_maps = make_core_inputs(x, ve, cos, sin, Wq, Wk, Wv, Wo, Wg)
    res = run_bass_kernel_spmd(nc, in_maps, list(range(8)), trace=_trace)
    outs = [np.asarray(r["out"], np.float32) for r in res.results]
    full = np.zeros((B, T, C), np.float32)
    for c in range(8):
        full[c // N_KV_HEAD] += outs[c]
    if _trace:
        kernel.last_results = res
    return full
